# revision 15
# baseline (speedup 1.0000x reference)
"""Bidirectional GQA self-attention (B=4, T=2048, C=2048, 16 q-heads /
4 kv-heads, RoPE) on 8 Trainium2 NeuronCores.

Sharding: row-data-parallel over (batch, token-half): core c handles batch
c//2, query tokens [ (c%2)*1024, (c%2)*1024+1024 ).  Each core computes the
full K/V for its batch (duplicated across the 2 cores of a batch - no
collectives, K/V stay in SBUF), all 16 heads of attention for its 1024
query tokens, and its [1024, 2048] slice of the final projection.

Key layout/eng decisions vs the fp32r baseline:
 - all matmul operands are bf16 (PE rate is identical to fp32r at N>=256,
   but DMA bytes, SBUF footprint and DVE element costs halve; accuracy
   budget ~0.5% << 2e-2 gate).
 - keys/values for the core's batch are stored own-half-first (host-side
   token permutation, cos/sin permuted identically) so all 8 cores run an
   identical program; softmax is permutation-invariant over keys.
 - A1 computes K (4 psum banks) and V (the other 4) in separate passes per
   512-token quarter so psum drain (rope / copies) overlaps the next pass.
 - A2 (q projection) and phase B (attention) are software-pipelined per
   head: emit A2(h+1) before B(h) so the PE never waits on rope/exp.
 - phase-B denominator: bf16 chunk-sum tree on DVE + gpsimd
   partition_all_reduce (no PE ones-matmuls, no psum bank for d).
 - exp on ACT reads [128,1024] f32 psum, writes bf16; ACT stays off the
   critical path (~290us vs PE ~340us in the merged A2+B section).
"""
import sys

sys.path.insert(0, "/opt/trn_rl_repo")

import numpy as np

import concourse.bass as bass
import concourse.mybir as mybir
import concourse.tile as tile
from concourse import bacc, bass_isa
from concourse.bass_utils import run_bass_kernel_spmd

B, T, C = 4, 2048, 2048
NH, NKV, HD = 16, 4, 128
REP = NH // NKV
TQ = 1024            # query tokens per core
NCORES = 8
SCALE = 1.0 / np.sqrt(HD)

BF = mybir.dt.bfloat16
F32 = mybir.dt.float32
MULT = mybir.AluOpType.mult
ADD = mybir.AluOpType.add
EXP = mybir.ActivationFunctionType.Exp
RADD = bass_isa.ReduceOp.add

# stream_shuffle mask: swap 16-halves within each 32-partition quadrant
SWAP16 = [(i + 16) % 32 for i in range(32)]

NCK = C // 128        # 16 contraction chunks over C
NQ_T = T // 512       # 4 token quarters of the full batch
NQ_Q = TQ // 512      # 2 token quarters of the own half
NTK = T // 128        # 16 key chunks


def _build(repeat=1, denom="mm", phases="full"):
    nc = bacc.Bacc("TRN2", target_bir_lowering=False, debug=False)

    xt = nc.dram_tensor("xt", [C, T], BF, kind="ExternalInput")
    wq = nc.dram_tensor("wq", [C, NH * HD], BF, kind="ExternalInput")
    wkv = nc.dram_tensor("wkv", [C, 2 * NKV * HD], BF, kind="ExternalInput")
    wp = nc.dram_tensor("wp", [C, C], BF, kind="ExternalInput")
    csq = nc.dram_tensor("csq", [128, TQ], BF, kind="ExternalInput")
    ssq = nc.dram_tensor("ssq", [128, TQ], BF, kind="ExternalInput")
    csk = nc.dram_tensor("csk", [128, T], BF, kind="ExternalInput")
    ssk = nc.dram_tensor("ssk", [128, T], BF, kind="ExternalInput")
    out = nc.dram_tensor("out", [TQ, C], F32, kind="ExternalOutput")

    import contextlib

    with tile.TileContext(nc) as tc:
        rep_cm = tc.For_i(0, repeat, 1) if repeat > 1 else contextlib.nullcontext()
        with (
            rep_cm,
            tc.tile_pool(name="cst", bufs=1) as cstp,
            tc.tile_pool(name="kv", bufs=1) as kvp,
            tc.tile_pool(name="yt", bufs=1) as ytp,
            tc.tile_pool(name="xo", bufs=1) as xop,
            tc.tile_pool(name="wqp", bufs=1) as wqp,
        ):
            k_tiles = [kvp.tile([128, T], BF, tag=f"k{m}", name=f"kT{m}")
                       for m in range(NKV)]
            v_tiles = [kvp.tile([128, 512], BF, tag=f"v{i}", name=f"vT{i}")
                       for i in range(NTK)]
            y_tiles = [ytp.tile([128, TQ], BF, tag=f"y{h}", name=f"yT{h}")
                       for h in range(NH)]
            csq_t = cstp.tile([128, TQ], BF, tag="csq")
            ssq_t = cstp.tile([128, TQ], BF, tag="ssq")

            # dst = t*CS + shuffle16(t*SS), all bf16 (DVE 2x mode)
            def rope_from_psum(rpp, dst_ap, ps_ap, cs_ap, ss_ap):
                tmp = rpp.tile([128, 512], BF, tag="ropeT")
                nc.scalar.copy(tmp[:], ps_ap)
                bb = rpp.tile([128, 512], BF, tag="ropeB")
                cc = rpp.tile([128, 512], BF, tag="ropeC")
                nc.vector.tensor_tensor(dst_ap, tmp[:], cs_ap, MULT)
                nc.vector.tensor_tensor(bb[:], tmp[:], ss_ap, MULT)
                nc.vector.stream_shuffle(cc[:], bb[:], SWAP16)
                nc.vector.tensor_tensor(dst_ap, dst_ap, cc[:], ADD)

            wq_cur = [None] * NCK

            def prefetch_wq(wset):
                for ck in range(NCK):
                    t = wqp.tile([128, 512], BF, tag=f"wq{ck}",
                                 name=f"wqt{wset}_{ck}")
                    wq_cur[ck] = t
                    nc.sync.dma_start(
                        t[:], wq[ck * 128:(ck + 1) * 128,
                                 wset * 512:(wset + 1) * 512])

            # ---- phase A1: K/V for the full batch (token-permuted: own
            # half first), K via rope into k_tiles, V into v_tiles. -------
            a1_cm = (
                tc.tile_pool(name="a1w", bufs=1),
                tc.tile_pool(name="a1x", bufs=32),
                tc.tile_pool(name="rp1", bufs=2),
                tc.tile_pool(name="psA", bufs=1, space="PSUM"),
            )
            wkvp, xtp, rp1, psA = (a1_cm[0].__enter__(), a1_cm[1].__enter__(),
                                   a1_cm[2].__enter__(), a1_cm[3].__enter__())
            csk_t = wkvp.tile([128, T], BF, tag="csk")
            ssk_t = wkvp.tile([128, T], BF, tag="ssk")
            x_q = [[None] * NCK for _ in range(NQ_T)]
            for q in range(NQ_Q):
                for ck in range(NCK):
                    x_q[q][ck] = xop.tile([128, 512], BF, tag=f"xo{q}_{ck}",
                                          name=f"xo{q}_{ck}")
            wkv_tiles = [wkvp.tile([128, 1024], BF, tag=f"wkv{ck}",
                                   name=f"wkv{ck}") for ck in range(NCK)]
            # qtr-0 inputs first so the PE can start ~1us in
            for ck in range(NCK):
                nc.sync.dma_start(wkv_tiles[ck][:],
                                  wkv[ck * 128:(ck + 1) * 128, :])
                nc.sync.dma_start(x_q[0][ck][:],
                                  xt[ck * 128:(ck + 1) * 128, 0:512])
            nc.sync.dma_start(csk_t[:], csk[:])
            nc.sync.dma_start(ssk_t[:], ssk[:])
            nc.sync.dma_start(csq_t[:], csq[:])
            nc.sync.dma_start(ssq_t[:], ssq[:])
            for q in range(1, NQ_T):
                for ck in range(NCK):
                    if q >= NQ_Q:
                        x_q[q][ck] = xtp.tile([128, 512], BF, tag="xot",
                                              name=f"xt{q}_{ck}")
                    nc.sync.dma_start(
                        x_q[q][ck][:],
                        xt[ck * 128:(ck + 1) * 128, q * 512:(q + 1) * 512])
            if phases != "a1":
                prefetch_wq(0)

            for q in range(NQ_T):
                # K pass (psum banks 0-3)
                k_ps = [psA.tile([128, 512], F32, tag=f"pk{m}",
                                 name=f"kps{m}") for m in range(NKV)]
                for ck in range(NCK):
                    for m in range(NKV):
                        nc.tensor.matmul(
                            k_ps[m][:],
                            wkv_tiles[ck][:, m * 128:(m + 1) * 128],
                            x_q[q][ck][:],
                            start=(ck == 0), stop=(ck == NCK - 1))
                for m in range(NKV):
                    rope_from_psum(
                        rp1, k_tiles[m][:, q * 512:(q + 1) * 512],
                        k_ps[m][:], csk_t[:, q * 512:(q + 1) * 512],
                        ssk_t[:, q * 512:(q + 1) * 512])
                # V pass (psum banks 4-7)
                v_ps = [psA.tile([128, 512], F32, tag=f"pv{tv}",
                                 name=f"vps{tv}") for tv in range(4)]
                for ck in range(NCK):
                    for tv in range(4):
                        nc.tensor.matmul(
                            v_ps[tv][:],
                            x_q[q][ck][:, tv * 128:(tv + 1) * 128],
                            wkv_tiles[ck][:, 512:1024],
                            start=(ck == 0), stop=(ck == NCK - 1))
                for tv in range(4):
                    nc.scalar.copy(v_tiles[q * 4 + tv][:], v_ps[tv][:])

            for cm in reversed(a1_cm):
                cm.__exit__(None, None, None)

            # ---- phase A2+B: per-head q projection + attention,
            # software-pipelined (emit A2(h+1) before B(h)). --------------
            with (
                tc.tile_pool(name="cw", bufs=1) as cwp,
                tc.tile_pool(name="qp", bufs=3) as qp,
                tc.tile_pool(name="pp", bufs=3) as pp,
                tc.tile_pool(name="dn", bufs=2) as dnp,
                tc.tile_pool(name="rp2", bufs=1) as rp2,
            ):
                psB_cm = tc.tile_pool(name="psB", bufs=1, space="PSUM")
                psB = psB_cm.__enter__()
                wp_tiles = [[None] * NCK for _ in range(2)]

                def prefetch_wp(nh):
                    for ck in range(NCK):
                        t = cwp.tile([128, 1024], BF, tag=f"wp{ck}",
                                     name=f"wpt{nh}_{ck}",
                                     bufs=2 if ck >= 8 else 1)
                        wp_tiles[nh][ck] = t
                        nc.sync.dma_start(
                            t[:], wp[ck * 128:(ck + 1) * 128,
                                     nh * 1024:(nh + 1) * 1024])

                q_cur = [None] * NH
                q_bufs = 2
                if denom == "mm":
                    ones = dnp.tile([128, 1], BF, tag="ones", bufs=1)
                    nc.vector.memset(ones[:], 1.0)
                    q_bufs = 1

                def emit_a2(h):
                    wset, mm = divmod(h, 4)
                    if mm == 0 and wset > 0:
                        prefetch_wq(wset)
                    qt = qp.tile([128, TQ], BF, tag="qt", name=f"qT{h}")
                    q_cur[h] = qt
                    for qq in range(NQ_Q):
                        q_ps = psB.tile([128, 512], F32, tag="q",
                                        bufs=q_bufs)
                        for ck in range(NCK):
                            nc.tensor.matmul(
                                q_ps[:],
                                wq_cur[ck][:, mm * 128:(mm + 1) * 128],
                                x_q[qq][ck][:],
                                start=(ck == 0), stop=(ck == NCK - 1))
                        rope_from_psum(
                            rp2, qt[:, qq * 512:(qq + 1) * 512], q_ps[:],
                            csq_t[:, qq * 512:(qq + 1) * 512],
                            ssq_t[:, qq * 512:(qq + 1) * 512])

                def emit_b(h):
                    g = h // REP
                    qt = q_cur[h]
                    for tqc in range(NQ_Q):
                        y_ps = psB.tile([128, 512], F32, tag="y", bufs=2)
                        acc = None
                        p_prev = None
                        for kp in range(NTK // 2):
                            s_ps = psB.tile([128, 1024], F32, tag="s",
                                            bufs=2)
                            for j in range(2):
                                kc = kp * 2 + j
                                nc.tensor.matmul(
                                    s_ps[:, j * 512:(j + 1) * 512],
                                    k_tiles[g][:, kc * 128:(kc + 1) * 128],
                                    qt[:, tqc * 512:(tqc + 1) * 512],
                                    start=True, stop=True)
                            p_t = pp.tile([128, 1024], BF, tag="p")
                            nc.scalar.activation(p_t[:], s_ps[:], EXP)
                            for j in range(2):
                                kc = kp * 2 + j
                                nc.tensor.matmul(
                                    y_ps[:],
                                    v_tiles[kc][:, g * 128:(g + 1) * 128],
                                    p_t[:, j * 512:(j + 1) * 512],
                                    start=(kc == 0), stop=(kc == NTK - 1))
                            if kp == 0:
                                p_prev = p_t
                            elif kp == 1:
                                acc = dnp.tile([128, 1024], BF, tag="acc")
                                nc.vector.tensor_tensor(
                                    acc[:], p_prev[:], p_t[:], ADD)
                            else:
                                nc.vector.tensor_tensor(
                                    acc[:], acc[:], p_t[:], ADD)
                        if denom == "gpsimd":
                            accf = dnp.tile([128, 512], F32, tag="accf")
                            nc.vector.tensor_tensor(
                                accf[:], acc[:, 0:512], acc[:, 512:1024],
                                ADD)
                            dbc = dnp.tile([128, 512], F32, tag="dbc")
                            nc.gpsimd.partition_all_reduce(
                                dbc[:], accf[:], 128, RADD)
                            rd = dnp.tile([128, 512], F32, tag="rd")
                            nc.vector.reciprocal(rd[:], dbc[:])
                        else:
                            d_ps = psB.tile([1, 512], F32, tag="d", bufs=1)
                            for j in range(2):
                                nc.tensor.matmul(
                                    d_ps[:], ones[:],
                                    acc[:, j * 512:(j + 1) * 512],
                                    start=(j == 0), stop=(j == 1))
                            rd1 = dnp.tile([1, 512], F32, tag="rd1", bufs=1)
                            nc.vector.reciprocal(rd1[:], d_ps[:])
                            rd = dnp.tile([128, 512], F32, tag="rd", bufs=1)
                            nc.gpsimd.partition_broadcast(rd[:], rd1[:])
                        nc.vector.tensor_tensor(
                            y_tiles[h][:, tqc * 512:(tqc + 1) * 512],
                            y_ps[:], rd[:], MULT)

                if phases != "a1":
                    emit_a2(0)
                    for h in range(NH):
                        if h + 1 < NH:
                            emit_a2(h + 1)
                        if phases == "full" and h == 7:
                            prefetch_wp(0)
                        if phases == "full" and h == 11:
                            prefetch_wp(1)
                        emit_b(h)

                psB_cm.__exit__(None, None, None)

                # ---- phase C: projection -------------------------------
                with (
                    tc.tile_pool(name="co", bufs=3) as cop,
                    tc.tile_pool(name="psC", bufs=3, space="PSUM") as psC,
                ):
                    cks = list(range(8, NCK)) + list(range(8))
                    for nh in range(2 if phases == "full" else 0):
                        for mt in range(TQ // 128):
                            for nn in range(2):
                                o_ps = psC.tile([128, 512], F32, tag="o")
                                for i, ck in enumerate(cks):
                                    nc.tensor.matmul(
                                        o_ps[:],
                                        y_tiles[ck][:,
                                                    mt * 128:(mt + 1) * 128],
                                        wp_tiles[nh][ck][:,
                                                         nn * 512:(nn + 1) * 512],
                                        start=(i == 0), stop=(i == NCK - 1))
                                o_t = cop.tile([128, 512], F32, tag="ot")
                                nc.scalar.copy(o_t[:], o_ps[:])
                                nc.sync.dma_start(
                                    out[mt * 128:(mt + 1) * 128,
                                        nh * 1024 + nn * 512:
                                        nh * 1024 + (nn + 1) * 512],
                                    o_t[:])
    return nc


_NC_CACHE = None


def _get_nc(repeat=1, denom="gpsimd", phases="full"):
    global _NC_CACHE
    if _NC_CACHE is None:
        _NC_CACHE = {}
    key = (repeat, denom, phases)
    if key not in _NC_CACHE:
        nc = _build(repeat, denom, phases)
        nc.compile()
        _NC_CACHE[key] = nc
    return _NC_CACHE[key]


def _head_perm():
    """col permutation within one head: new[qd*32 + e*16 + s] = old[2*(qd*16+s)+e]"""
    idx = np.empty(128, np.int64)
    for qd in range(4):
        for e in range(2):
            for s in range(16):
                idx[qd * 32 + e * 16 + s] = 2 * (qd * 16 + s) + e
    return idx


def _bf16(a):
    from ml_dtypes import bfloat16
    return np.ascontiguousarray(a.astype(bfloat16))


def make_in_maps(x, freqs_cis, w_qkv, w_proj):
    x = np.asarray(x, dtype=np.float32)
    freqs_cis = np.asarray(freqs_cis, dtype=np.float32)
    w_qkv = np.asarray(w_qkv, dtype=np.float32)
    w_proj = np.asarray(w_proj, dtype=np.float32)

    hp = _head_perm()
    qperm = np.concatenate([h * 128 + hp for h in range(NH)])
    kperm = np.concatenate([h * 128 + hp for h in range(NKV)])
    wq = _bf16(w_qkv[:, :NH * HD][:, qperm])
    wk = w_qkv[:, NH * HD:NH * HD + NKV * HD][:, kperm]
    wv = w_qkv[:, NH * HD + NKV * HD:]
    wkv = _bf16(np.concatenate([wk, wv], axis=1))
    wp = _bf16(w_proj)

    cos = np.ascontiguousarray(freqs_cis[:, :, 0].T)  # [64, T]
    sin = np.ascontiguousarray(freqs_cis[:, :, 1].T)
    pair = np.empty(128, np.int64)
    sgn = np.empty(128, np.float32)
    for qd in range(4):
        for e in range(2):
            for s in range(16):
                row = qd * 32 + e * 16 + s
                pair[row] = qd * 16 + s
                sgn[row] = 1.0 if e == 0 else -1.0
    CS = cos[pair]                  # [128, T]
    SS = sin[pair] * sgn[:, None]   # [128, T]

    xT = [np.ascontiguousarray(x[b].T) for b in range(B)]

    in_maps = []
    for c in range(NCORES):
        b, h = divmod(c, 2)
        own = slice(h * TQ, (h + 1) * TQ)
        oth = slice((1 - h) * TQ, (2 - h) * TQ)
        # tokens permuted own-half-first for K/V (and their cos/sin);
        # queries (csq/ssq) stay in own-half order.
        in_maps.append({
            "xt": _bf16(np.concatenate([xT[b][:, own], xT[b][:, oth]], 1)),
            "wq": wq, "wkv": wkv, "wp": wp,
            "csq": _bf16(CS[:, own] * np.float32(SCALE)),
            "ssq": _bf16(SS[:, own] * np.float32(SCALE)),
            "csk": _bf16(np.concatenate([CS[:, own], CS[:, oth]], 1)),
            "ssk": _bf16(np.concatenate([SS[:, own], SS[:, oth]], 1)),
        })
    return in_maps


def kernel(x, freqs_cis, w_qkv, w_proj):
    nc = _get_nc()
    in_maps = make_in_maps(x, freqs_cis, w_qkv, w_proj)
    res = run_bass_kernel_spmd(nc, in_maps, list(range(NCORES)))
    full = np.empty((B, T, C), np.float32)
    for c in range(NCORES):
        b, h = divmod(c, 2)
        full[b, h * TQ:(h + 1) * TQ, :] = res.results[c]["out"]
    return full
